# revision 34
# baseline (speedup 1.0000x reference)
"""Trainium2 Bass kernel for the cross-attention block nn_CA_54889682043704.

Reference computation (B=4, C=512, N=M=4096, da=128):
    q = w_qk @ x                      [B, da, N]
    k = w_qk @ y                      [B, da, M]
    v = w_v @ y + b_v                 [B, C, M]
    attn = softmax((q^T k) / sqrt(da), axis=M)
    x_s = v @ attn^T                  [B, C, N]
    out = relu(BN(w_t @ x_s + b_t)) transposed to [B, N, C]

Sharding: (batch b, query-half h) -> 8 cores, collective-free. Each core
computes the full attention for 2048 queries of one batch element.

Per-core dataflow (all matmuls bf16 inputs with fp32 PSUM accumulation):
    k   [da=128p, 4096]   and  vT  32 x [128p(m), 512(c)] = y-tile^T @ w_v^T,
        computed per 512-key block pipelined with the y DMA stream
    q   [da=128p, 2048]
    per n-tile (512 queries):
      per m-chunk (128 keys):
        E^T [m128p, n512] = k-slice^T @ q-slice          (energy, transposed)
        P^T = exp(E^T)  (ACT; softmax max-subtraction skipped -- energies are
                         O(1) so exp is safe; softmax is shift-invariant)
        pacc += P^T  (DVE; per-partition partial of the softmax denominator)
        S[c,n] += vT-slice^T @ P^T   (4 c-chunks, accumulated over m in PSUM)
      denom[1,n] = ones^T @ pacc  (PE partition-reduction)
      denom -> [n,1] via tiny K=1 transpose-matmuls; recip = 1/denom (DVE)
      T[n128p, c512] = S-slice^T @ W_eff^T  (output projection, transposed
                       into the final [n, c] layout; BN scale folded in)
      out = relu(T * recip[n] + bias_eff[c])  (ACT scale + DVE add/relu)
    The denom/T/epilogue tail of n-tile i is emitted interleaved into the
    middle of n-tile i+1's m-loop so the PE never drains at tile boundaries.

Host-side folding: b_v never reaches the device (softmax rows sum to 1 =>
w_t@b_v joins b_t); BN gamma/var folds into w_t (W_eff) and bias_eff.
"""

import sys

for _p in ("/opt/trn_rl_repo", "/root/.axon_site/_ro/trn_rl_repo"):
    if _p not in sys.path:
        sys.path.append(_p)

import math
import numpy as np
import ml_dtypes

import concourse.bacc as bacc
import concourse.bass as bass
import concourse.mybir as mybir
from concourse import tile
from concourse.bass_utils import run_bass_kernel_spmd

B, C, N, M = 4, 512, 4096, 4096
DA = 128
NCORES = 8
NL = N // 2            # queries per core
CCH = C // 128         # 4 channel chunks
MCH = M // 128         # 32 key chunks
NTILES = NL // 512     # 4 query tiles per core
BN_EPS = 1e-5
SCALE = 1.0 / math.sqrt(DA)
WEARLY = CCH * DA + CCH * C + CCH * DA   # wk | wv | wq packed cols

BF16 = mybir.dt.bfloat16
F32 = mybir.dt.float32
NP_BF16 = ml_dtypes.bfloat16
PSUM = bass.MemorySpace.PSUM


def build_program():
    nc = bacc.Bacc("TRN2", target_bir_lowering=False, debug=False,
                   num_devices=NCORES)

    xc_d = nc.dram_tensor("xc", [CCH, 128, NL], BF16, kind="ExternalInput").ap()
    yc_d = nc.dram_tensor("yc", [CCH, 128, M], BF16, kind="ExternalInput").ap()
    we_d = nc.dram_tensor("wearly", [128, WEARLY], BF16,
                          kind="ExternalInput").ap()
    wt_d = nc.dram_tensor("wlate", [128, CCH * C], BF16,
                          kind="ExternalInput").ap()
    bias_d = nc.dram_tensor("biasb", [128, C], F32, kind="ExternalInput").ap()
    out_d = nc.dram_tensor("out", [NL, C], F32, kind="ExternalOutput").ap()

    with tile.TileContext(nc) as tc:
        with (
            tc.tile_pool(name="persist", bufs=1) as wp,
            tc.tile_pool(name="vtp", bufs=MCH) as vtp,
            tc.tile_pool(name="ptp", bufs=16) as ptp,
            tc.tile_pool(name="accp", bufs=2) as accp,
            tc.tile_pool(name="ssb", bufs=8) as ssbp,
            tc.tile_pool(name="ep", bufs=4) as ep,
            tc.tile_pool(name="psA", bufs=3, space=PSUM) as psA,
            tc.tile_pool(name="psS", bufs=4, space=PSUM) as psS,
            tc.tile_pool(name="psD", bufs=1, space=PSUM) as psD,
        ):
            ones128 = wp.tile([128, 1], F32, tag="ones128", name="ones128")
            nc.vector.memset(ones128[:], 1.0)
            zb = wp.tile([128, 1], F32, tag="zb", name="zb")
            nc.vector.memset(zb[:], 0.0)
            # HAM warmup: the PE idles ~6us while the first y blocks stream
            # in and would start cold (1.2GHz). A dozen dummy matmuls on a
            # zeroed scratch tile keep the activity monitor busy so the real
            # k/vT matmuls start at the full 2.4GHz clock.
            scratch = wp.tile([128, 512], BF16, tag="scratch", name="scratch")
            nc.vector.memset(scratch[:], 0.0)
            for i in range(12):
                dps = psA.tile([128, 512], F32, tag="et", name=f"warm{i}")
                nc.tensor.matmul(dps[:], lhsT=scratch[:, 0:128],
                                 rhs=scratch[:], start=True, stop=True)

            # wk alone first (tiny, gates the k pipeline), then wv|wq packed
            wearly = wp.tile([128, WEARLY], BF16, tag="wearly", name="wearly")
            nc.sync.dma_start(out=wearly[:, :CCH * DA],
                              in_=we_d[:, :CCH * DA])
            nc.gpsimd.dma_start(out=wearly[:, CCH * DA:],
                                in_=we_d[:, CCH * DA:])
            wk = [wearly[:, ci * DA:(ci + 1) * DA] for ci in range(CCH)]
            wv = [wearly[:, CCH * DA + ci * C:CCH * DA + (ci + 1) * C]
                  for ci in range(CCH)]
            _q0 = CCH * DA + CCH * C
            wq = [wearly[:, _q0 + ci * DA:_q0 + (ci + 1) * DA]
                  for ci in range(CCH)]

            # y on the sync DGE ring, ordered by first use: the first half in
            # fine 512-col blocks so the k/vT pipeline starts ASAP, the
            # second half in big low-issue-overhead transfers
            yt = [wp.tile([128, M], BF16, tag=f"y{i}", name=f"y{i}")
                  for i in range(CCH)]
            for mb in range(4):
                for i in range(CCH):
                    # block 0 split across both DGE rings so all four
                    # c-chunks land ASAP and the first k matmuls can start
                    eng = nc.gpsimd if (mb == 0 and i >= 2) else nc.sync
                    eng.dma_start(
                        out=yt[i][:, mb * 512:(mb + 1) * 512],
                        in_=yc_d[i, :, mb * 512:(mb + 1) * 512])
            for i in range(CCH):
                nc.sync.dma_start(
                    out=yt[i][:, M // 2:],
                    in_=yc_d[i, :, M // 2:])

            # x on the gpsimd ring (parallel with y), then late weights
            xt = []
            for i in range(CCH):
                t = wp.tile([128, NL], BF16, tag=f"x{i}", name=f"x{i}")
                nc.gpsimd.dma_start(out=t[:], in_=xc_d[i])
                xt.append(t)
            wlate = wp.tile([128, CCH * C], BF16, tag="wlate", name="wlate")
            nc.gpsimd.dma_start(out=wlate[:], in_=wt_d)
            wt = [wlate[:, ci * C:(ci + 1) * C] for ci in range(CCH)]
            biasb = wp.tile([128, C], F32, tag="biasb", name="biasb")
            nc.gpsimd.dma_start(out=biasb[:], in_=bias_d)

            q_sb = wp.tile([128, NL], BF16, tag="qsb", name="qsb")
            k_sb = wp.tile([128, M], BF16, tag="ksb", name="ksb")

            # ---- k and vT per 512-key block, pipelined with the y stream
            vt = [None] * MCH
            for mb in range(M // 512):
                ps = psA.tile([128, 512], F32, tag="et", name=f"kps{mb}")
                for ci in range(CCH):
                    nc.tensor.matmul(ps[:], lhsT=wk[ci],
                                     rhs=yt[ci][:, mb * 512:(mb + 1) * 512],
                                     start=(ci == 0), stop=(ci == CCH - 1))
                nc.vector.tensor_copy(k_sb[:, mb * 512:(mb + 1) * 512], ps[:])
                for mj in range(mb * 4, mb * 4 + 4):
                    psv = psA.tile([128, C], F32, tag="et", name=f"vps{mj}")
                    for ci in range(CCH):
                        nc.tensor.matmul(psv[:],
                                         lhsT=yt[ci][:, mj * 128:(mj + 1) * 128],
                                         rhs=wv[ci],
                                         start=(ci == 0), stop=(ci == CCH - 1))
                    v = vtp.tile([128, C], BF16, tag="vt", name=f"vt{mj}")
                    nc.vector.tensor_copy(v[:], psv[:])
                    vt[mj] = v
                if mb == 3:
                    # q = (scale*w_qk) @ x -> [128, NL]; emitted here so the
                    # PE has fill work while the second half of y streams in
                    for nt in range(NL // 512):
                        ps = psA.tile([128, 512], F32, tag="et",
                                      name=f"qps{nt}")
                        for ci in range(CCH):
                            nc.tensor.matmul(
                                ps[:], lhsT=wq[ci],
                                rhs=xt[ci][:, nt * 512:(nt + 1) * 512],
                                start=(ci == 0), stop=(ci == CCH - 1))
                        nc.vector.tensor_copy(
                            q_sb[:, nt * 512:(nt + 1) * 512], ps[:])

            # ---- attention + output projection, one 512-query tile at a time
            # tail(0) emits the denominator reduction; tail(1..4) emit one
            # output chunk each, spread across the next tile's m-loop so the
            # ACT/DVE epilogue work never backs up the exp pipeline
            def make_tail(nt, pacc, s_sb):
                last = nt == NTILES - 1
                state = {}

                def tail0():
                    dn_ps = psD.tile([1, 512], F32, tag="dn", name=f"dn{nt}")
                    nc.tensor.matmul(dn_ps[:], lhsT=ones128[:], rhs=pacc[:],
                                     start=True, stop=True)
                    dn_sb = ep.tile([1, 512], F32, tag="dnsb", name=f"dnsb{nt}")
                    if last:
                        nc.scalar.copy(dn_sb[:], dn_ps[:])
                    else:
                        nc.vector.tensor_copy(dn_sb[:], dn_ps[:])
                    dt_ps = psD.tile([128, 4], F32, tag="dn", name=f"dt{nt}")
                    for g in range(4):
                        nc.tensor.matmul(dt_ps[:, g:g + 1],
                                         lhsT=dn_sb[0:1, g * 128:(g + 1) * 128],
                                         rhs=ones128[0:1, 0:1],
                                         start=True, stop=True)
                    recip = ep.tile([128, 4], F32, tag="recip",
                                    name=f"recip{nt}")
                    nc.vector.reciprocal(recip[:], dt_ps[:])
                    state["recip"] = recip

                def tail_g(g):
                    n0 = nt * 512
                    recip = state["recip"]
                    if True:
                        # mid-kernel tails keep T off the m-loop's PSUM slots
                        # and the epilogue off the busy DVE; the last tail
                        # uses the (now free) fast path for minimum latency
                        if last:
                            t_ps = psA.tile([128, C], F32, tag="et",
                                            name=f"t{nt}_{g}")
                        else:
                            t_ps = psD.tile([128, C], F32, tag="dn",
                                            name=f"t{nt}_{g}")
                        for ci in range(CCH):
                            nc.tensor.matmul(
                                t_ps[:],
                                lhsT=s_sb[ci][:, g * 128:(g + 1) * 128],
                                rhs=wt[ci],
                                start=(ci == 0), stop=(ci == CCH - 1))
                        u = ep.tile([128, C], F32, tag="u", name=f"u{nt}_{g}")
                        nc.scalar.mul(u[:], t_ps[:], mul=recip[:, g:g + 1])
                        o = ep.tile([128, C], F32, tag="o", name=f"o{nt}_{g}")
                        nc.vector.tensor_tensor(o[:], u[:], biasb[:],
                                                op=mybir.AluOpType.add)
                        nc.vector.tensor_scalar_max(o[:], o[:], 0.0)
                        deng = nc.gpsimd if (last and g % 2) else nc.sync
                        deng.dma_start(
                            out=out_d[n0 + g * 128:n0 + (g + 1) * 128, :],
                            in_=o[:])

                return [tail0] + [lambda g=g: tail_g(g) for g in range(4)]

            pending_tails = []
            for nt in range(NTILES):
                n0 = nt * 512
                s_ps = [psS.tile([128, 512], F32, tag="s", name=f"s{nt}_{ci}")
                        for ci in range(CCH)]
                pacc = accp.tile([128, 512], F32, tag="pacc", name=f"pacc{nt}")
                for mj in range(MCH):
                    et = psA.tile([128, 512], F32, tag="et", name=f"et{nt}_{mj}")
                    nc.tensor.matmul(et[:],
                                     lhsT=k_sb[:, mj * 128:(mj + 1) * 128],
                                     rhs=q_sb[:, n0:n0 + 512],
                                     start=True, stop=True)
                    pt = ptp.tile([128, 512], BF16, tag="pt", name=f"pt{nt}_{mj}")
                    nc.scalar.activation(pt[:], et[:],
                                         mybir.ActivationFunctionType.Exp,
                                         bias=zb[:])
                    if mj == 0:
                        nc.vector.tensor_copy(pacc[:], pt[:])
                    else:
                        nc.vector.tensor_tensor(pacc[:], pacc[:], pt[:],
                                                op=mybir.AluOpType.add)
                    for ci in range(CCH):
                        nc.tensor.matmul(s_ps[ci][:],
                                         lhsT=vt[mj][:, ci * 128:(ci + 1) * 128],
                                         rhs=pt[:],
                                         start=(mj == 0), stop=(mj == MCH - 1))
                    if pending_tails and mj in (3, 9, 15, 21, 27):
                        pending_tails.pop(0)()

                # S -> SBUF (bf16) for use as matmul stationaries
                s_sb = []
                for ci in range(CCH):
                    t = ssbp.tile([128, 512], BF16, tag="ssb",
                                  name=f"ssb{nt}_{ci}")
                    if nt == NTILES - 1 and ci >= 2:
                        # split the final evacuation across DVE and ACT to
                        # shorten the end-of-kernel critical chain
                        nc.scalar.copy(t[:], s_ps[ci][:])
                    else:
                        nc.vector.tensor_copy(t[:], s_ps[ci][:])
                    s_sb.append(t)
                pending_tails = make_tail(nt, pacc, s_sb)
            for t in pending_tails:
                t()

    nc.compile()
    return nc


_PROG = None


def _get_prog():
    global _PROG
    if _PROG is None:
        _PROG = build_program()
    return _PROG


def _prep_in_maps(x, y, w_qk, w_v, b_v, w_t, b_t, gamma, beta, run_mean,
                  run_var):
    f32 = lambda a: np.asarray(a, dtype=np.float32)
    x, y = f32(x), f32(y)
    w_qk, w_v, b_v = f32(w_qk), f32(w_v), f32(b_v)
    w_t, b_t = f32(w_t), f32(b_t)
    gamma, beta = f32(gamma), f32(beta)
    run_mean, run_var = f32(run_mean), f32(run_var)

    inv = gamma / np.sqrt(run_var + BN_EPS)
    # b_v folded through attention (softmax rows sum to 1), BN folded into w_t
    b_t_eff = w_t @ b_v + b_t
    bias_eff = b_t_eff * inv + beta - run_mean * inv
    weffT = (w_t * inv[:, None]).T          # [c, o]

    def tob(a):
        return np.ascontiguousarray(a).astype(NP_BF16)

    wk_p = tob(w_qk.T)                      # [C, DA] -> 4 chunks [128, 128]
    wv_p = tob(w_v.T)                       # [C, C]
    wq_p = tob((w_qk * SCALE).T)
    # packed as [128, wk(4*128) | wv(4*512) | wq(4*128)] with chunks side by side
    def chunks(a, w):
        return [a[ci * 128:(ci + 1) * 128] for ci in range(CCH)]

    wearly = np.concatenate(
        chunks(wk_p, DA) + chunks(wv_p, C) + chunks(wq_p, DA), axis=1)
    wlate = np.concatenate(chunks(tob(weffT), C), axis=1)
    bias_h = np.ascontiguousarray(
        np.broadcast_to(bias_eff.astype(np.float32), (128, C)))

    in_maps = []
    for core in range(NCORES):
        b, h = divmod(core, 2)
        in_maps.append({
            "xc": tob(x[b][:, h * NL:(h + 1) * NL]).reshape(CCH, 128, NL),
            "yc": tob(y[b]).reshape(CCH, 128, M),
            "wearly": wearly, "wlate": wlate, "biasb": bias_h,
        })
    return in_maps


def run(trace=False, **inputs):
    nc = _get_prog()
    in_maps = _prep_in_maps(**inputs)
    res = run_bass_kernel_spmd(nc, in_maps, core_ids=list(range(NCORES)),
                               trace=trace)
    out = np.empty((B, N, C), np.float32)
    for core in range(NCORES):
        b, h = divmod(core, 2)
        out[b, h * NL:(h + 1) * NL, :] = res.results[core]["out"]
    return out, res


def kernel(**inputs):
    out, _ = run(trace=False, **inputs)
    return out


# revision 35
# speedup vs baseline: 1.0092x; 1.0092x over previous
"""Trainium2 Bass kernel for the cross-attention block nn_CA_54889682043704.

Reference computation (B=4, C=512, N=M=4096, da=128):
    q = w_qk @ x                      [B, da, N]
    k = w_qk @ y                      [B, da, M]
    v = w_v @ y + b_v                 [B, C, M]
    attn = softmax((q^T k) / sqrt(da), axis=M)
    x_s = v @ attn^T                  [B, C, N]
    out = relu(BN(w_t @ x_s + b_t)) transposed to [B, N, C]

Sharding: (batch b, query-half h) -> 8 cores, collective-free. Each core
computes the full attention for 2048 queries of one batch element.

Per-core dataflow (all matmuls bf16 inputs with fp32 PSUM accumulation):
    k   [da=128p, 4096]   and  vT  32 x [128p(m), 512(c)] = y-tile^T @ w_v^T,
        computed per 512-key block pipelined with the y DMA stream
    q   [da=128p, 2048]
    per n-tile (512 queries):
      per m-chunk (128 keys):
        E^T [m128p, n512] = k-slice^T @ q-slice          (energy, transposed)
        P^T = exp(E^T)  (ACT; softmax max-subtraction skipped -- energies are
                         O(1) so exp is safe; softmax is shift-invariant)
        pacc += P^T  (DVE; per-partition partial of the softmax denominator)
        S[c,n] += vT-slice^T @ P^T   (4 c-chunks, accumulated over m in PSUM)
      denom[1,n] = ones^T @ pacc  (PE partition-reduction)
      denom -> [n,1] via tiny K=1 transpose-matmuls; recip = 1/denom (DVE)
      T[n128p, c512] = S-slice^T @ W_eff^T  (output projection, transposed
                       into the final [n, c] layout; BN scale folded in)
      out = relu(T * recip[n] + bias_eff[c])  (ACT scale + DVE add/relu)
    The denom/T/epilogue tail of n-tile i is emitted interleaved into the
    middle of n-tile i+1's m-loop so the PE never drains at tile boundaries.

Host-side folding: b_v never reaches the device (softmax rows sum to 1 =>
w_t@b_v joins b_t); BN gamma/var folds into w_t (W_eff) and bias_eff.
"""

import sys

for _p in ("/opt/trn_rl_repo", "/root/.axon_site/_ro/trn_rl_repo"):
    if _p not in sys.path:
        sys.path.append(_p)

import math
import numpy as np
import ml_dtypes

import concourse.bacc as bacc
import concourse.bass as bass
import concourse.mybir as mybir
from concourse import tile
from concourse.bass_utils import run_bass_kernel_spmd

B, C, N, M = 4, 512, 4096, 4096
DA = 128
NCORES = 8
NL = N // 2            # queries per core
CCH = C // 128         # 4 channel chunks
MCH = M // 128         # 32 key chunks
NTILES = NL // 512     # 4 query tiles per core
BN_EPS = 1e-5
SCALE = 1.0 / math.sqrt(DA)
WEARLY = CCH * DA + CCH * C + CCH * DA   # wk | wv | wq packed cols

BF16 = mybir.dt.bfloat16
F32 = mybir.dt.float32
NP_BF16 = ml_dtypes.bfloat16
PSUM = bass.MemorySpace.PSUM


def build_program():
    nc = bacc.Bacc("TRN2", target_bir_lowering=False, debug=False,
                   num_devices=NCORES)

    xc_d = nc.dram_tensor("xc", [CCH, 128, NL], BF16, kind="ExternalInput").ap()
    yc_d = nc.dram_tensor("yc", [CCH, 128, M], BF16, kind="ExternalInput").ap()
    we_d = nc.dram_tensor("wearly", [128, WEARLY], BF16,
                          kind="ExternalInput").ap()
    wt_d = nc.dram_tensor("wlate", [128, CCH * C], BF16,
                          kind="ExternalInput").ap()
    bias_d = nc.dram_tensor("biasb", [128, C], F32, kind="ExternalInput").ap()
    out_d = nc.dram_tensor("out", [NL, C], F32, kind="ExternalOutput").ap()

    with tile.TileContext(nc) as tc:
        with (
            tc.tile_pool(name="persist", bufs=1) as wp,
            tc.tile_pool(name="vtp", bufs=MCH) as vtp,
            tc.tile_pool(name="ptp", bufs=16) as ptp,
            tc.tile_pool(name="accp", bufs=2) as accp,
            tc.tile_pool(name="ssb", bufs=8) as ssbp,
            tc.tile_pool(name="ep", bufs=4) as ep,
            tc.tile_pool(name="psA", bufs=3, space=PSUM) as psA,
            tc.tile_pool(name="psS", bufs=4, space=PSUM) as psS,
            tc.tile_pool(name="psD", bufs=1, space=PSUM) as psD,
        ):
            ones128 = wp.tile([128, 1], F32, tag="ones128", name="ones128")
            nc.vector.memset(ones128[:], 1.0)
            zb = wp.tile([128, 1], F32, tag="zb", name="zb")
            nc.vector.memset(zb[:], 0.0)

            # wk alone first (tiny, gates the k pipeline), then wv|wq packed
            wearly = wp.tile([128, WEARLY], BF16, tag="wearly", name="wearly")
            nc.sync.dma_start(out=wearly[:, :CCH * DA],
                              in_=we_d[:, :CCH * DA])
            nc.gpsimd.dma_start(out=wearly[:, CCH * DA:],
                                in_=we_d[:, CCH * DA:])
            wk = [wearly[:, ci * DA:(ci + 1) * DA] for ci in range(CCH)]
            wv = [wearly[:, CCH * DA + ci * C:CCH * DA + (ci + 1) * C]
                  for ci in range(CCH)]
            _q0 = CCH * DA + CCH * C
            wq = [wearly[:, _q0 + ci * DA:_q0 + (ci + 1) * DA]
                  for ci in range(CCH)]

            # y on the sync DGE ring, ordered by first use: the first half in
            # fine 512-col blocks so the k/vT pipeline starts ASAP, the
            # second half in big low-issue-overhead transfers
            yt = [wp.tile([128, M], BF16, tag=f"y{i}", name=f"y{i}")
                  for i in range(CCH)]
            for mb in range(4):
                for i in range(CCH):
                    # block 0 split across both DGE rings so all four
                    # c-chunks land ASAP and the first k matmuls can start
                    eng = nc.gpsimd if (mb == 0 and i >= 2) else nc.sync
                    eng.dma_start(
                        out=yt[i][:, mb * 512:(mb + 1) * 512],
                        in_=yc_d[i, :, mb * 512:(mb + 1) * 512])
            for i in range(CCH):
                nc.sync.dma_start(
                    out=yt[i][:, M // 2:],
                    in_=yc_d[i, :, M // 2:])

            # x on the gpsimd ring (parallel with y), then late weights
            xt = []
            for i in range(CCH):
                t = wp.tile([128, NL], BF16, tag=f"x{i}", name=f"x{i}")
                nc.gpsimd.dma_start(out=t[:], in_=xc_d[i])
                xt.append(t)
            wlate = wp.tile([128, CCH * C], BF16, tag="wlate", name="wlate")
            nc.gpsimd.dma_start(out=wlate[:], in_=wt_d)
            wt = [wlate[:, ci * C:(ci + 1) * C] for ci in range(CCH)]
            biasb = wp.tile([128, C], F32, tag="biasb", name="biasb")
            nc.gpsimd.dma_start(out=biasb[:], in_=bias_d)

            q_sb = wp.tile([128, NL], BF16, tag="qsb", name="qsb")
            k_sb = wp.tile([128, M], BF16, tag="ksb", name="ksb")

            # ---- k and vT per 512-key block, pipelined with the y stream
            vt = [None] * MCH
            for mb in range(M // 512):
                ps = psA.tile([128, 512], F32, tag="et", name=f"kps{mb}")
                for ci in range(CCH):
                    nc.tensor.matmul(ps[:], lhsT=wk[ci],
                                     rhs=yt[ci][:, mb * 512:(mb + 1) * 512],
                                     start=(ci == 0), stop=(ci == CCH - 1))
                nc.vector.tensor_copy(k_sb[:, mb * 512:(mb + 1) * 512], ps[:])
                for mj in range(mb * 4, mb * 4 + 4):
                    psv = psA.tile([128, C], F32, tag="et", name=f"vps{mj}")
                    for ci in range(CCH):
                        nc.tensor.matmul(psv[:],
                                         lhsT=yt[ci][:, mj * 128:(mj + 1) * 128],
                                         rhs=wv[ci],
                                         start=(ci == 0), stop=(ci == CCH - 1))
                    v = vtp.tile([128, C], BF16, tag="vt", name=f"vt{mj}")
                    nc.vector.tensor_copy(v[:], psv[:])
                    vt[mj] = v
                if mb == 3:
                    # q = (scale*w_qk) @ x -> [128, NL]; emitted here so the
                    # PE has fill work while the second half of y streams in
                    for nt in range(NL // 512):
                        ps = psA.tile([128, 512], F32, tag="et",
                                      name=f"qps{nt}")
                        for ci in range(CCH):
                            nc.tensor.matmul(
                                ps[:], lhsT=wq[ci],
                                rhs=xt[ci][:, nt * 512:(nt + 1) * 512],
                                start=(ci == 0), stop=(ci == CCH - 1))
                        nc.vector.tensor_copy(
                            q_sb[:, nt * 512:(nt + 1) * 512], ps[:])

            # ---- attention + output projection, one 512-query tile at a time
            # tail(0) emits the denominator reduction; tail(1..4) emit one
            # output chunk each, spread across the next tile's m-loop so the
            # ACT/DVE epilogue work never backs up the exp pipeline
            def make_tail(nt, pacc, s_sb):
                last = nt == NTILES - 1
                state = {}

                def tail0():
                    dn_ps = psD.tile([1, 512], F32, tag="dn", name=f"dn{nt}")
                    nc.tensor.matmul(dn_ps[:], lhsT=ones128[:], rhs=pacc[:],
                                     start=True, stop=True)
                    dn_sb = ep.tile([1, 512], F32, tag="dnsb", name=f"dnsb{nt}")
                    if last:
                        nc.scalar.copy(dn_sb[:], dn_ps[:])
                    else:
                        nc.vector.tensor_copy(dn_sb[:], dn_ps[:])
                    dt_ps = psD.tile([128, 4], F32, tag="dn", name=f"dt{nt}")
                    for g in range(4):
                        nc.tensor.matmul(dt_ps[:, g:g + 1],
                                         lhsT=dn_sb[0:1, g * 128:(g + 1) * 128],
                                         rhs=ones128[0:1, 0:1],
                                         start=True, stop=True)
                    recip = ep.tile([128, 4], F32, tag="recip",
                                    name=f"recip{nt}")
                    nc.vector.reciprocal(recip[:], dt_ps[:])
                    state["recip"] = recip

                def tail_g(g):
                    n0 = nt * 512
                    recip = state["recip"]
                    if True:
                        # mid-kernel tails keep T off the m-loop's PSUM slots
                        # and the epilogue off the busy DVE; the last tail
                        # uses the (now free) fast path for minimum latency
                        if last:
                            t_ps = psA.tile([128, C], F32, tag="et",
                                            name=f"t{nt}_{g}")
                        else:
                            t_ps = psD.tile([128, C], F32, tag="dn",
                                            name=f"t{nt}_{g}")
                        for ci in range(CCH):
                            nc.tensor.matmul(
                                t_ps[:],
                                lhsT=s_sb[ci][:, g * 128:(g + 1) * 128],
                                rhs=wt[ci],
                                start=(ci == 0), stop=(ci == CCH - 1))
                        u = ep.tile([128, C], F32, tag="u", name=f"u{nt}_{g}")
                        nc.scalar.mul(u[:], t_ps[:], mul=recip[:, g:g + 1])
                        o = ep.tile([128, C], F32, tag="o", name=f"o{nt}_{g}")
                        nc.vector.tensor_tensor(o[:], u[:], biasb[:],
                                                op=mybir.AluOpType.add)
                        nc.vector.tensor_scalar_max(o[:], o[:], 0.0)
                        deng = nc.gpsimd if (last and g % 2) else nc.sync
                        deng.dma_start(
                            out=out_d[n0 + g * 128:n0 + (g + 1) * 128, :],
                            in_=o[:])

                return [tail0] + [lambda g=g: tail_g(g) for g in range(4)]

            pending_tails = []
            for nt in range(NTILES):
                n0 = nt * 512
                s_ps = [psS.tile([128, 512], F32, tag="s", name=f"s{nt}_{ci}")
                        for ci in range(CCH)]
                pacc = accp.tile([128, 512], F32, tag="pacc", name=f"pacc{nt}")
                for mj in range(MCH):
                    et = psA.tile([128, 512], F32, tag="et", name=f"et{nt}_{mj}")
                    nc.tensor.matmul(et[:],
                                     lhsT=k_sb[:, mj * 128:(mj + 1) * 128],
                                     rhs=q_sb[:, n0:n0 + 512],
                                     start=True, stop=True)
                    pt = ptp.tile([128, 512], BF16, tag="pt", name=f"pt{nt}_{mj}")
                    nc.scalar.activation(pt[:], et[:],
                                         mybir.ActivationFunctionType.Exp,
                                         bias=zb[:])
                    if mj == 0:
                        nc.vector.tensor_copy(pacc[:], pt[:])
                    else:
                        nc.vector.tensor_tensor(pacc[:], pacc[:], pt[:],
                                                op=mybir.AluOpType.add)
                    for ci in range(CCH):
                        nc.tensor.matmul(s_ps[ci][:],
                                         lhsT=vt[mj][:, ci * 128:(ci + 1) * 128],
                                         rhs=pt[:],
                                         start=(mj == 0), stop=(mj == MCH - 1))
                    if pending_tails and mj in (3, 9, 15, 21, 27):
                        pending_tails.pop(0)()

                # S -> SBUF (bf16) for use as matmul stationaries
                s_sb = []
                for ci in range(CCH):
                    t = ssbp.tile([128, 512], BF16, tag="ssb",
                                  name=f"ssb{nt}_{ci}")
                    if nt == NTILES - 1 and ci >= 2:
                        # split the final evacuation across DVE and ACT to
                        # shorten the end-of-kernel critical chain
                        nc.scalar.copy(t[:], s_ps[ci][:])
                    else:
                        nc.vector.tensor_copy(t[:], s_ps[ci][:])
                    s_sb.append(t)
                pending_tails = make_tail(nt, pacc, s_sb)
            for t in pending_tails:
                t()

    nc.compile()
    return nc


_PROG = None


def _get_prog():
    global _PROG
    if _PROG is None:
        _PROG = build_program()
    return _PROG


def _prep_in_maps(x, y, w_qk, w_v, b_v, w_t, b_t, gamma, beta, run_mean,
                  run_var):
    f32 = lambda a: np.asarray(a, dtype=np.float32)
    x, y = f32(x), f32(y)
    w_qk, w_v, b_v = f32(w_qk), f32(w_v), f32(b_v)
    w_t, b_t = f32(w_t), f32(b_t)
    gamma, beta = f32(gamma), f32(beta)
    run_mean, run_var = f32(run_mean), f32(run_var)

    inv = gamma / np.sqrt(run_var + BN_EPS)
    # b_v folded through attention (softmax rows sum to 1), BN folded into w_t
    b_t_eff = w_t @ b_v + b_t
    bias_eff = b_t_eff * inv + beta - run_mean * inv
    weffT = (w_t * inv[:, None]).T          # [c, o]

    def tob(a):
        return np.ascontiguousarray(a).astype(NP_BF16)

    wk_p = tob(w_qk.T)                      # [C, DA] -> 4 chunks [128, 128]
    wv_p = tob(w_v.T)                       # [C, C]
    wq_p = tob((w_qk * SCALE).T)
    # packed as [128, wk(4*128) | wv(4*512) | wq(4*128)] with chunks side by side
    def chunks(a, w):
        return [a[ci * 128:(ci + 1) * 128] for ci in range(CCH)]

    wearly = np.concatenate(
        chunks(wk_p, DA) + chunks(wv_p, C) + chunks(wq_p, DA), axis=1)
    wlate = np.concatenate(chunks(tob(weffT), C), axis=1)
    bias_h = np.ascontiguousarray(
        np.broadcast_to(bias_eff.astype(np.float32), (128, C)))

    in_maps = []
    for core in range(NCORES):
        b, h = divmod(core, 2)
        in_maps.append({
            "xc": tob(x[b][:, h * NL:(h + 1) * NL]).reshape(CCH, 128, NL),
            "yc": tob(y[b]).reshape(CCH, 128, M),
            "wearly": wearly, "wlate": wlate, "biasb": bias_h,
        })
    return in_maps


def run(trace=False, **inputs):
    nc = _get_prog()
    in_maps = _prep_in_maps(**inputs)
    res = run_bass_kernel_spmd(nc, in_maps, core_ids=list(range(NCORES)),
                               trace=trace)
    out = np.empty((B, N, C), np.float32)
    for core in range(NCORES):
        b, h = divmod(core, 2)
        out[b, h * NL:(h + 1) * NL, :] = res.results[core]["out"]
    return out, res


def kernel(**inputs):
    out, _ = run(trace=False, **inputs)
    return out
